# revision 35
# baseline (speedup 1.0000x reference)
"""Causal multi-head self-attention with RoPE on 8 TRN2 NeuronCores.

Problem: b=4, s=2048, d_model=1024, 16 heads, d_k=64, fp32 I/O.

Sharding: core c = (batch b = c//2, head-half g = c%2). Each core computes the
8 heads of one head-half for one batch element, applies its slice of the
output projection, and returns a partial [2048, 1024]; the host sums the two
partials per batch (the tensor-parallel all-reduce done on host).

On-core pipeline (all matmuls bf16, fp32 PSUM accumulation):
  1. Q^T/K^T projections into [d_head, seq] layout (lhsT = W^T chunk,
     rhs = x^T chunk), V projection into [seq, d_head] layout.
  2. RoPE: Q' = P2^T Q via a matmul with a constant pair-swap(+negate) matrix,
     then Qrot = Q*cosE + Q'*sinE elementwise (cos/sin tables host-computed
     from token_positions, laid out to match the [d, seq] tiles).
  3. Scores transposed: St[k, q] = Krot^T chunk . Qrot (two heads packed per
     128-partition tile, two concurrent K=64 matmuls at row groups 0/64
     writing the two halves of one [128, 1024] PSUM tile). One exp per
     k-block via ScalarE from PSUM with scale=1/8 (no max subtraction:
     scores are O(5) here, exp is safe in fp32), causal triangle zeroed with
     one gpsimd affine_select per diagonal block; off-diagonal invalid
     columns are never computed (q-range restricted per k block).
  4. attn @ V without transposing P: out^T[d, q] = V_aug^T . Pt with
     V_aug = [V | ones]; the ones column makes row 64 of the PSUM the softmax
     denominators. Broadcast denominators across partitions with a tiny
     ones-matmul, reciprocal on VectorE, normalize on GpSimd during eviction.
  5. o_proj: out[seq, d_model] partial = concat^T tiles . Wo^T slice.

Emission order interleaves projection of head-pair t+1 with attention of
head-pair t so ScalarE exp work overlaps TensorE projection work, keeping
the PE stream dense (HAM stays un-throttled).
"""

import numpy as np
import ml_dtypes

bf16 = ml_dtypes.bfloat16

N_HEADS = 16
THETA = 10000.0
B, S, D = 4, 2048, 1024
DK = D // N_HEADS          # 64
DH = D // 2                # 512 dims per core (8 heads)
P = 128
NKC = D // P               # 8 contraction chunks for projections
NSG = S // 512             # 4 seq groups of 512
NST = S // P               # 16 seq tiles of 128
NPAIR = DH // P            # 4 head-pair tiles per core
VW = 66                    # per-head stride in interleaved V tile

_CACHE = {}


def _build_program():
    import concourse.tile as tile
    from concourse import bacc, mybir

    nc = bacc.Bacc("TRN2", target_bir_lowering=False, debug=False, num_devices=1)
    dt = mybir.dt

    xt_d = nc.dram_tensor("xt", [D, S], dt.bfloat16, kind="ExternalInput")
    wq_d = nc.dram_tensor("wq", [D, DH], dt.bfloat16, kind="ExternalInput")
    wk_d = nc.dram_tensor("wk", [D, DH], dt.bfloat16, kind="ExternalInput")
    wv_d = nc.dram_tensor("wv", [D, DH], dt.bfloat16, kind="ExternalInput")
    wo_d = nc.dram_tensor("wo", [DH, D], dt.bfloat16, kind="ExternalInput")
    cos_d = nc.dram_tensor("cosE", [P, S], dt.bfloat16, kind="ExternalInput")
    sin_d = nc.dram_tensor("sinE", [P, S], dt.bfloat16, kind="ExternalInput")
    p2_d = nc.dram_tensor("p2", [P, P], dt.bfloat16, kind="ExternalInput")
    bc_d = nc.dram_tensor("bcsel", [P, 256], dt.bfloat16, kind="ExternalInput")
    out_d = nc.dram_tensor("out", [S, D], dt.float32, kind="ExternalOutput")

    EXP = mybir.ActivationFunctionType.Exp

    with tile.TileContext(nc) as tc:
        with tc.tile_pool(name="const", bufs=1) as cst, \
             tc.tile_pool(name="persist", bufs=1) as per, \
             tc.tile_pool(name="work", bufs=4) as wkp, \
             tc.tile_pool(name="ev", bufs=2) as evp, \
             tc.tile_pool(name="up", bufs=4, space="PSUM") as pup:

            # ---- constants; DMA order chosen so the first matmuls start early
            # xt as [128, 512] tiles so early seq groups unblock fast;
            # interleave wq with xt_sg0 so the first projection chain streams
            wq = []
            xt = [[None] * NSG for _ in range(NKC)]
            wk_, wv = [], []
            for sg in range(NSG):
                for kc in range(NKC):
                    if sg == 0:
                        t = cst.tile([P, DH], dt.bfloat16, tag=f"wq{kc}",
                                     name=f"wq{kc}")
                        nc.sync.dma_start(t[:], wq_d.ap()[P * kc:P * (kc + 1), :])
                        wq.append(t)
                    t = cst.tile([P, 512], dt.bfloat16, tag=f"xt{kc}_{sg}",
                                 name=f"xt{kc}_{sg}")
                    dma_eng = (nc.scalar, nc.gpsimd)[kc % 2]
                    dma_eng.dma_start(
                        t[:], xt_d.ap()[P * kc:P * (kc + 1),
                                        512 * sg:512 * (sg + 1)])
                    xt[kc][sg] = t
                if sg == 0:
                    for kc in range(NKC):
                        t = cst.tile([P, DH], dt.bfloat16, tag=f"wk{kc}",
                                     name=f"wk{kc}")
                        nc.scalar.dma_start(t[:], wk_d.ap()[P * kc:P * (kc + 1), :])
                        wk_.append(t)
                if sg == 1:
                    cosE = cst.tile([P, S], dt.bfloat16, tag="cos")
                    nc.scalar.dma_start(cosE[:], cos_d.ap())
                    sinE = cst.tile([P, S], dt.bfloat16, tag="sin")
                    nc.scalar.dma_start(sinE[:], sin_d.ap())
                    p2 = cst.tile([P, P], dt.bfloat16, tag="p2")
                    nc.sync.dma_start(p2[:], p2_d.ap())
                if sg == 2:
                    for kc in range(NKC):
                        t = cst.tile([P, DH], dt.bfloat16, tag=f"wv{kc}",
                                     name=f"wv{kc}")
                        nc.gpsimd.dma_start(t[:], wv_d.ap()[P * kc:P * (kc + 1), :])
                        wv.append(t)
            bcsel = cst.tile([P, 256], dt.bfloat16, tag="bc")
            nc.sync.dma_start(bcsel[:], bc_d.ap())
            wo = []
            for t_i in range(NPAIR):
                t = cst.tile([P, D], dt.bfloat16, tag=f"wo{t_i}", name=f"wo{t_i}")
                nc.sync.dma_start(t[:], wo_d.ap()[P * t_i:P * (t_i + 1), :])
                wo.append(t)

            qrot = [per.tile([P, S], dt.bfloat16, tag=f"qrot{t_i}",
                             name=f"qrot{t_i}") for t_i in range(NPAIR)]
            krot = [per.tile([P, S], dt.bfloat16, tag=f"krot{t_i}",
                             name=f"krot{t_i}") for t_i in range(NPAIR)]
            vil = [per.tile([P, 8 * VW], dt.bfloat16, tag=f"v{m}",
                            name=f"vil{m}") for m in range(NST)]
            conc = [per.tile([P, S], dt.bfloat16, tag=f"conc{t_i}",
                             name=f"conc{t_i}") for t_i in range(NPAIR)]

            def qk_proj(t_i):
                """Q^T and K^T projections + RoPE for head-pair tile t_i."""
                for sg in range(NSG):
                    for (w_tiles, rot) in ((wq, qrot), (wk_, krot)):
                        ps = pup.tile([P, 1024], dt.float32, tag="u")
                        for kc in range(NKC):
                            nc.tensor.matmul(
                                ps[:, 0:512],
                                w_tiles[kc][:, P * t_i:P * (t_i + 1)],
                                xt[kc][sg][:], start=(kc == 0),
                                stop=(kc == NKC - 1))
                        qsb = wkp.tile([P, 512], dt.bfloat16, tag="qsb")
                        nc.vector.tensor_copy(qsb[:], ps[:, 0:512])
                        nc.tensor.matmul(ps[:, 512:1024], p2[:], qsb[:],
                                         start=True, stop=True)
                        tmp1 = wkp.tile([P, 512], dt.bfloat16, tag="tmp1")
                        nc.vector.tensor_mul(tmp1[:], qsb[:],
                                             cosE[:, 512 * sg:512 * (sg + 1)])
                        q2sb = wkp.tile([P, 512], dt.bfloat16, tag="q2sb")
                        nc.vector.tensor_copy(q2sb[:], ps[:, 512:1024])
                        tmp2 = wkp.tile([P, 512], dt.bfloat16, tag="tmp2")
                        nc.vector.tensor_mul(tmp2[:], q2sb[:],
                                             sinE[:, 512 * sg:512 * (sg + 1)])
                        nc.vector.tensor_add(
                            rot[t_i][:, 512 * sg:512 * (sg + 1)],
                            tmp1[:], tmp2[:])

            def v_proj(m):
                ps = pup.tile([P, 1024], dt.float32, tag="u", name="pshalf")[:, 0:512]
                sg, mo = divmod(m, 4)
                for kc in range(NKC):
                    nc.tensor.matmul(ps[:], xt[kc][sg][:, P * mo:P * (mo + 1)],
                                     wv[kc][:, :],
                                     start=(kc == 0), stop=(kc == NKC - 1))
                v3 = vil[m][:].rearrange("p (h c) -> p h c", c=VW)
                nc.gpsimd.memset(v3[:, :, 64:65], 1.0)
                nc.vector.tensor_copy(v3[:, :, 0:64],
                                      ps[:].rearrange("p (h c) -> p h c", c=64))

            def attention(t_i, pre_gq=None):
                cA, cB = VW * (2 * t_i), VW * (2 * t_i + 1)
                # staging for unnormalized head outputs + sums row:
                # [65, 4096], column = 1024*gq + 512*head_in_pair + q_local
                ev = evp.tile([65, 2 * S], dt.bfloat16, tag="ev", name=f"ev{t_i}")
                for gq in range(NSG):
                    if pre_gq is not None:
                        pre_gq(gq)
                    avAB = pup.tile([P, 1024], dt.float32, tag="u", name="avAB")
                    nki = 4 * gq + 4
                    for ki in range(nki - 1, -1, -1):
                        joff = max(0, P * ki - 512 * gq)
                        width = 512 - joff
                        qs = slice(512 * gq + joff, 512 * (gq + 1))
                        ks = slice(P * ki, P * (ki + 1))
                        sAB = pup.tile([P, 1024], dt.float32, tag="u", name="sAB")
                        nc.tensor.matmul(sAB[:, 0:width], krot[t_i][0:64, ks],
                                         qrot[t_i][0:64, qs],
                                         start=True, stop=True)
                        nc.tensor.matmul(sAB[:, 512:512 + width],
                                         krot[t_i][64:128, ks],
                                         qrot[t_i][64:128, qs],
                                         start=True, stop=True)
                        ptAB = wkp.tile([P, 1024], dt.bfloat16, tag="pt")
                        s3 = sAB[:].rearrange("p (two c) -> p two c", two=2)
                        pt3 = ptAB[:].rearrange("p (two c) -> p two c", two=2)
                        nc.scalar.activation(pt3[:, :, 0:width], s3[:, :, 0:width],
                                             EXP, bias=0.0, scale=0.125)
                        if ki >= 4 * gq:  # diagonal: zero upper triangle, both heads
                            nc.gpsimd.affine_select(
                                pt3[:, :, 0:P], pt3[:, :, 0:P],
                                pattern=[[0, 2], [1, P]],
                                compare_op=mybir.AluOpType.is_ge, fill=0.0,
                                base=0, channel_multiplier=-1)
                        nc.tensor.matmul(avAB[0:65, joff:512],
                                         vil[ki][:, cA:cA + 65],
                                         ptAB[:, 0:width],
                                         start=(ki == nki - 1), stop=(ki == 0))
                        nc.tensor.matmul(avAB[0:65, 512 + joff:1024],
                                         vil[ki][:, cB:cB + 65],
                                         ptAB[:, 512:512 + width],
                                         start=(ki == nki - 1), stop=(ki == 0))
                    nc.vector.tensor_copy(ev[:, 1024 * gq:1024 * gq + 512],
                                          avAB[0:65, 0:512])
                    nc.scalar.copy(ev[:, 1024 * gq + 512:1024 * (gq + 1)],
                                   avAB[0:65, 512:1024])
                    if t_i == NPAIR - 1:
                        gs = slice(512 * gq, 512 * (gq + 1))
                        bcT = pup.tile([64, 1024], dt.float32, tag="u", name="bcT")
                        nc.tensor.matmul(bcT[:, 0:512], bcsel[64:65, 0:64],
                                         ev[64:65, 1024 * gq:1024 * gq + 512],
                                         start=True, stop=True)
                        nc.tensor.matmul(bcT[:, 512:1024], bcsel[64:65, 0:64],
                                         ev[64:65, 1024 * gq + 512:1024 * (gq + 1)],
                                         start=True, stop=True)
                        rcT = evp.tile([64, 1024], dt.float32, tag="rcT",
                                       name="rcT", bufs=1)
                        nc.vector.reciprocal_approx_fast(rcT[:], bcT[:])
                        nc.gpsimd.tensor_mul(conc[t_i][0:64, gs],
                                             ev[0:64, 1024 * gq:1024 * gq + 512],
                                             rcT[:, 0:512])
                        nc.gpsimd.tensor_mul(
                            conc[t_i][64:128, gs],
                            ev[0:64, 1024 * gq + 512:1024 * (gq + 1)],
                            rcT[:, 512:1024])
                if t_i == NPAIR - 1:
                    return
                # per-pair normalization, off the PSUM critical path
                rcA = evp.tile([64, S], dt.float32, tag="rcA", name=f"rcA{t_i}",
                               bufs=1)
                rcB = evp.tile([64, S], dt.float32, tag="rcB", name=f"rcB{t_i}",
                               bufs=1)
                for gq in range(NSG):
                    gs = slice(512 * gq, 512 * (gq + 1))
                    bcAB = pup.tile([64, 1024], dt.float32, tag="u", name="bcAB")
                    nc.tensor.matmul(bcAB[:, 0:512], bcsel[64:65, 0:64],
                                     ev[64:65, 1024 * gq:1024 * gq + 512],
                                     start=True, stop=True)
                    nc.tensor.matmul(bcAB[:, 512:1024], bcsel[64:65, 0:64],
                                     ev[64:65, 1024 * gq + 512:1024 * (gq + 1)],
                                     start=True, stop=True)
                    nc.vector.reciprocal_approx_fast(rcA[:, gs], bcAB[:, 0:512])
                    nc.vector.reciprocal_approx_fast(rcB[:, gs], bcAB[:, 512:1024])
                for gq in range(NSG):
                    gs = slice(512 * gq, 512 * (gq + 1))
                    nc.gpsimd.tensor_mul(conc[t_i][0:64, gs],
                                         ev[0:64, 1024 * gq:1024 * gq + 512],
                                         rcA[:, gs])
                    nc.gpsimd.tensor_mul(conc[t_i][64:128, gs],
                                         ev[0:64, 1024 * gq + 512:1024 * (gq + 1)],
                                         rcB[:, gs])

            # ---- interleaved emission ------------------------------------------
            qk_proj(0)
            for m in range(4):
                v_proj(m)

            def v_filler(gq):
                # feed V projections for the NEXT q-group as PE filler
                for m in range(4 * (gq + 1), min(NST, 4 * (gq + 2))):
                    v_proj(m)

            for t_i in range(NPAIR):
                if t_i + 1 < NPAIR:
                    # emit next pair's projections before this pair's attention so
                    # the scheduler has dense PE work while ScalarE runs exp
                    qk_proj(t_i + 1)
                attention(t_i, pre_gq=v_filler if t_i == 0 else None)

            # ---- output projection ---------------------------------------------
            for m in range(NST):
                for ng in range(2):
                    ps = pup.tile([P, 1024], dt.float32, tag="u", name="pshalf")[:, 0:512]
                    for t_i in range(NPAIR):
                        nc.tensor.matmul(ps[:], conc[t_i][:, P * m:P * (m + 1)],
                                         wo[t_i][:, 512 * ng:512 * (ng + 1)],
                                         start=(t_i == 0), stop=(t_i == NPAIR - 1))
                    osb = wkp.tile([P, 512], dt.float32, tag="osb")
                    nc.vector.tensor_copy(osb[:], ps[:])
                    (nc.sync if m % 2 == 0 else nc.gpsimd).dma_start(
                        out_d.ap()[P * m:P * (m + 1), 512 * ng:512 * (ng + 1)],
                        osb[:])

    nc.compile()
    return nc


def _host_tables(token_positions):
    pos = np.asarray(token_positions).astype(np.float32)
    inv_freq = (THETA ** (-(np.arange(0, DK, 2, dtype=np.float32)) / DK))  # [32]
    ang = pos[:, None] * inv_freq[None, :]                                 # [s, 32]
    cos_t = np.cos(ang).T                                                  # [32, s]
    sin_t = np.sin(ang).T
    sel = (np.arange(P) % DK) // 2
    cosE = np.ascontiguousarray(cos_t[sel, :]).astype(bf16)                # [128, s]
    sinE = np.ascontiguousarray(sin_t[sel, :]).astype(bf16)

    p2 = np.zeros((P, P), dtype=np.float32)
    for i in range(P // 2):
        p2[2 * i + 1, 2 * i] = -1.0
        p2[2 * i, 2 * i + 1] = 1.0
    p2 = p2.astype(bf16)

    bcsel = np.zeros((P, 256), dtype=np.float32)
    bcsel[:, 0:64] = 1.0      # A selector: broadcast row to partitions 0..63
    bcsel[:, 192:256] = 1.0   # B selector: broadcast row to partitions 64..127
    bcsel = bcsel.astype(bf16)
    return cosE, sinE, p2, bcsel


def _in_maps(x, Wq, Wk, Wv, Wo, token_positions):
    cosE, sinE, p2, bcsel = _host_tables(token_positions)
    in_maps = []
    for c in range(8):
        b, g = c // 2, c % 2
        rows = slice(DH * g, DH * (g + 1))
        in_maps.append({
            "xt": np.ascontiguousarray(x[b].T).astype(bf16),
            "wq": np.ascontiguousarray(Wq[rows, :].T).astype(bf16),
            "wk": np.ascontiguousarray(Wk[rows, :].T).astype(bf16),
            "wv": np.ascontiguousarray(Wv[rows, :].T).astype(bf16),
            "wo": np.ascontiguousarray(Wo[:, rows].T).astype(bf16),
            "cosE": cosE, "sinE": sinE, "p2": p2, "bcsel": bcsel,
        })
    return in_maps


def kernel(in_features, Wq, Wk, Wv, Wo, token_positions):
    from concourse import bass_utils

    x = np.asarray(in_features, dtype=np.float32)
    Wq = np.asarray(Wq, dtype=np.float32)
    Wk = np.asarray(Wk, dtype=np.float32)
    Wv = np.asarray(Wv, dtype=np.float32)
    Wo = np.asarray(Wo, dtype=np.float32)

    if "nc" not in _CACHE:
        _CACHE["nc"] = _build_program()
    nc = _CACHE["nc"]

    in_maps = _in_maps(x, Wq, Wk, Wv, Wo, token_positions)
    res = bass_utils.run_bass_kernel_spmd(nc, in_maps, core_ids=list(range(8)))
    out = np.empty((B, S, D), dtype=np.float32)
    for b in range(B):
        out[b] = res.results[2 * b]["out"] + res.results[2 * b + 1]["out"]
    return out
